# revision 38
# baseline (speedup 1.0000x reference)
"""DGCN diffusion-graph-conv kernel for 8 Trainium2 NeuronCores.

Math (per the reference):
    support S = D^-1/2 (adj+I)^T D^-1/2  with D = diag(rowsum(adj+I))
    x_m = T_m(S) x0  (Chebyshev recurrence, K=3 -> m=0..3)
    out = sum_m x_m @ W_m + bias

Strategy (data-parallel over batch, 4 batches/core, mixed precision):
    Fold Chebyshev coefficients into the weights and expand the
    recurrence into explicit support powers (host-precomputed in fp32):
        V0 = W0 - W2, V1 = W1 - 3*W3, V2 = 2*W2, V3 = 4*W3
        U_m = x0 @ V_m                        (contracts feature dim)
        out = U0 + S*U1 + S^2*U2 + S^3*U3 + bias   (contracts nodes)
    U0 feeds the output undamped -> bf16 matmuls (fp8 measured 4e-2,
    over the gate).  U1..U3 and the S^m multiplies are damped by the
    contractive support spectrum -> fp8 e4m3 DoubleRow matmuls (2x PE
    rate).  The three S^m terms accumulate into one PSUM group, so
    there is no serial diffusion chain on the device at all.
    Power-of-2 scales keep fp8 operands in the normal range:
        x*8, V_m*32 (m>=1), S^m*2^14, U_m carried *8.

Schedule notes (v3): every DRAM image is partition-major AND column-
split so each startup DMA moves contiguous-per-partition lines (the
strided 512B-segment loads measured ~22 GB/s; contiguous ~300+);
startup DMAs ordered critical-first across the three DMA queues
(sync/scalar HWDGE, gpsimd SWDGE); x0R persistent; psum->fp8 converts
split between scalar ACT (512 cols) and vector DVE (256 cols); junk
matmuls pre-warm the HAM clock gate during the initial DMA wait.
"""

import numpy as np
import ml_dtypes

import concourse.bacc as bacc
import concourse.tile as tile
import concourse.mybir as mybir
from concourse.bass_utils import run_bass_kernel_spmd

F32 = mybir.dt.float32
BF16 = mybir.dt.bfloat16
F8 = mybir.dt.float8e4
DR = mybir.MatmulPerfMode.DoubleRow
ALU = mybir.AluOpType
AFT = mybir.ActivationFunctionType
E4M3 = ml_dtypes.float8_e4m3

N_CORES = 8
B, N, D = 32, 512, 768
BL = B // N_CORES          # local batches per core = 4
BN = BL * N                # local rows = 2048
RT = BN // 128             # 16 row tiles
NT = N // 128              # 4 node tiles per batch
DT = D // 128              # 6 feature k-subtiles
PW = 1536                  # batch-pair column width (2*768)
N_WARM = 13                # junk matmuls to pre-warm the PE clock

SX = 8.0                   # x fp8 pre-scale
SV = 32.0                  # V1..V3 fp8 pre-scale
SS = float(2 ** 14)        # S^m fp8 pre-scale
SU = 8.0                   # U_m carried *8 in fp8
C_U = SU / (SX * SV)       # psum(U123) -> 8*U_m     (= 1/32)
C_O = 1.0 / (SS * SU)      # psum(combine) -> out    (= 2^-17)


def _build_program():
    nc = bacc.Bacc("TRN2", target_bir_lowering=False, debug=False,
                   num_devices=N_CORES)
    # Partition-major images; V matrices pre-split into the 512- and
    # 256-wide column groups so every DMA is contiguous per partition.
    x0R_d = nc.dram_tensor("x0R", [128, RT, DT, 128], BF16,
                           kind="ExternalInput").ap()
    x8P_d = nc.dram_tensor("x8P", [128, RT, DT, 128], F8,
                           kind="ExternalInput").ap()
    v0A_d = nc.dram_tensor("v0A", [128, DT, 512], BF16,
                           kind="ExternalInput").ap()
    v0B_d = nc.dram_tensor("v0B", [128, DT, 256], BF16,
                           kind="ExternalInput").ap()
    v8A_d = nc.dram_tensor("v8A", [128, 3, DT, 512], F8,
                           kind="ExternalInput").ap()
    v8B_d = nc.dram_tensor("v8B", [128, 3, DT, 256], F8,
                           kind="ExternalInput").ap()
    s8P_d = nc.dram_tensor("s8P", [128, 3, NT, N], F8,
                           kind="ExternalInput").ap()
    # bias pre-broadcast on host so no SWDGE replicate DMA is needed
    bias_d = nc.dram_tensor("bias", [128, D], F32,
                            kind="ExternalInput").ap()
    out_d = nc.dram_tensor("out", [BN, D], F32, kind="ExternalOutput").ap()

    with tile.TileContext(nc) as tc:
        with (
            tc.tile_pool(name="const", bufs=1) as constp,
            tc.tile_pool(name="ut", bufs=6) as utp,
            tc.tile_pool(name="u0t", bufs=2) as u0p,
            tc.tile_pool(name="ost", bufs=3) as ostp,
            tc.tile_pool(name="psP", bufs=6, space="PSUM") as psP,
            tc.tile_pool(name="psH", bufs=2, space="PSUM") as psH,
        ):
            X8 = constp.tile([128, RT, DT, 128], F8, name="X8")
            X0R = constp.tile([128, RT, DT, 128], BF16, name="X0R")
            V8A = constp.tile([128, 3, DT, 512], F8, name="V8A")
            V8B = constp.tile([128, 3, DT, 256], F8, name="V8B")
            V0A = constp.tile([128, DT, 512], BF16, name="V0A")
            V0B = constp.tile([128, DT, 256], BF16, name="V0B")
            S8s = constp.tile([128, 3, NT, N], F8, name="S8s")
            biasb = constp.tile([128, D], F32, name="biasb")
            warm = constp.tile([128, 512], BF16, name="warm")

            # ---- startup DMAs ----
            # The um(0,3) phase re-streams ALL of V8[2] within its first
            # row-tile (~1us in), so V8[2] + X8[0] gate the start.  Put
            # the critical stream on BOTH HWDGE rings interleaved in
            # deadline order; everything not needed before ~40us goes on
            # the delayed gpsimd ring or the HWDGE tails.
            # scalar issues only 4 chunks — its queue must drain before
            # the first psum->fp8 ACT (~11us) or the ACTs (and the psum
            # recycling behind them) block on DMA lane-reuse waits.
            nc.scalar.dma_start(V8A[:, 2, 0:2, :], v8A_d[:, 2, 0:2, :])
            nc.sync.dma_start(X8[:, 0:1, :, :], x8P_d[:, 0:1, :, :])
            nc.scalar.dma_start(V8A[:, 2, 2:6, :], v8A_d[:, 2, 2:6, :])
            nc.sync.dma_start(V8B[:, 2, :, :], v8B_d[:, 2, :, :])
            nc.scalar.dma_start(V8B[:, 1, :, :], v8B_d[:, 1, :, :])
            nc.sync.dma_start(X8[:, 1:2, :, :], x8P_d[:, 1:2, :, :])
            nc.sync.dma_start(X8[:, 2:4, :, :], x8P_d[:, 2:4, :, :])
            nc.sync.dma_start(X8[:, 4:8, :, :], x8P_d[:, 4:8, :, :])
            nc.sync.dma_start(V8A[:, 1, :, :], v8A_d[:, 1, :, :])
            nc.sync.dma_start(V8A[:, 0, :, :], v8A_d[:, 0, :, :])
            nc.sync.dma_start(V8B[:, 0, :, :], v8B_d[:, 0, :, :])
            nc.sync.dma_start(S8s[:, :, :, :], s8P_d[:, :, :, :])
            nc.sync.dma_start(X8[:, 8:12, :, :], x8P_d[:, 8:12, :, :])
            nc.sync.dma_start(X8[:, 12:16, :, :], x8P_d[:, 12:16, :, :])
            nc.sync.dma_start(X0R[:, 8:12, :, :], x0R_d[:, 8:12, :, :])
            nc.sync.dma_start(X0R[:, 12:16, :, :], x0R_d[:, 12:16, :, :])
            # gpsimd (SWDGE): its bulk is not needed before ~40us, so
            # gate each DMA behind a junk-memset wall via tiny copies
            # (copy reads the stall tile -> RAW dep; DMA dst overlaps
            # the copy's dst -> WAW dep).  The scheduler cannot hoist
            # these DMAs into the critical 8-13us window.
            stall = constp.tile([128, 2048], F32, name="stall")
            nc.gpsimd.memset(stall[:], 0.0)
            nc.gpsimd.tensor_scalar_add(biasb[0:128, 0:4], stall[:, 0:4], 0.0)
            nc.gpsimd.memset(stall[:], 1.0)
            nc.gpsimd.memset(stall[:], 2.0)
            nc.gpsimd.tensor_scalar_add(V0A[0:128, 0, 0:4], stall[:, 0:4], 0.0)
            nc.gpsimd.tensor_scalar_add(V0B[0:128, 0, 0:4], stall[:, 4:8], 0.0)
            nc.gpsimd.memset(stall[:], 3.0)
            nc.gpsimd.tensor_scalar_add(
                X0R[0:128, 0, 0, 0:4], stall[:, 0:4], 0.0)
            nc.gpsimd.tensor_scalar_add(
                X0R[0:128, 4, 0, 0:4], stall[:, 4:8], 0.0)
            nc.gpsimd.dma_start(biasb[:], bias_d[:, :])
            nc.gpsimd.dma_start(V0A[:, :, :], v0A_d[:, :, :])
            nc.gpsimd.dma_start(V0B[:, :, :], v0B_d[:, :, :])
            nc.gpsimd.dma_start(X0R[:, 0:4, :, :], x0R_d[:, 0:4, :, :])
            nc.gpsimd.dma_start(X0R[:, 4:8, :, :], x0R_d[:, 4:8, :, :])

            # PE warm-up: junk matmuls during the initial DMA wait ramp
            # the tensor-engine p-state so real work starts at full
            # clock.  They use the psH pool (idle until combine, ~49us)
            # so the warm chain never blocks the real stream's psP ring.
            nc.vector.memset(warm[:], 0.0)
            for i in range(N_WARM):
                pw = psH.tile([128, 512], F32, name=f"warm{i}", tag="ps")
                nc.tensor.matmul(pw[:], warm[:, 0:128], warm[:],
                                 start=True, stop=True)

            def proj_u0(pair):
                """U0 (+bias) for batch pair (bf16 matmuls) -> u0b fp32."""
                u0b = u0p.tile([128, NT, PW], F32, name=f"u0b{pair}",
                               tag="u0")
                for bi in range(2):
                    b = 2 * pair + bi
                    for nt in range(NT):
                        rt = b * NT + nt
                        ps = psP.tile([128, 512], F32, name=f"pA{rt}",
                                      tag="ps")
                        ps2 = psP.tile([128, 512], F32, name=f"pB{rt}",
                                       tag="ps")
                        for t in range(DT):
                            nc.tensor.matmul(
                                ps[:], X0R[:, rt, t, :], V0A[:, t, :],
                                start=(t == 0), stop=(t == DT - 1))
                            nc.tensor.matmul(
                                ps2[:, 0:256], X0R[:, rt, t, :],
                                V0B[:, t, :],
                                start=(t == 0), stop=(t == DT - 1))
                        c0 = bi * D
                        nc.vector.tensor_add(
                            u0b[:, nt, c0:c0 + 512], ps[:], biasb[:, 0:512])
                        nc.vector.tensor_add(
                            u0b[:, nt, c0 + 512:c0 + D], ps2[:, 0:256],
                            biasb[:, 512:D])
                return u0b

            def proj_um(pair, m, dst):
                """One U_m phase (fp8 DoubleRow) for a batch pair -> fp8.

                psum->fp8 converts split: 512-chunk on scalar ACT,
                256-chunk on vector DVE, so neither engine paces the PE.
                """
                for bi in range(2):
                    b = 2 * pair + bi
                    for nt in range(NT):
                        rt = b * NT + nt
                        c0 = bi * D
                        ps = psP.tile([128, 512], F32,
                                      name=f"pU{rt}_{m}", tag="ps")
                        ps2 = psP.tile([128, 512], F32,
                                       name=f"pV{rt}_{m}", tag="ps")
                        for t in range(DT // 2):
                            xs = X8[:, rt, 2 * t:2 * t + 2, :]
                            nc.tensor.matmul(
                                ps[:], xs,
                                V8A[:, m - 1, 2 * t:2 * t + 2, :],
                                start=(t == 0), stop=(t == 2),
                                perf_mode=DR)
                            nc.tensor.matmul(
                                ps2[:, 0:256], xs,
                                V8B[:, m - 1, 2 * t:2 * t + 2, :],
                                start=(t == 0), stop=(t == 2),
                                perf_mode=DR)
                        nc.scalar.activation(
                            dst[:, nt, c0:c0 + 512], ps[:],
                            AFT.Copy, scale=C_U)
                        nc.vector.tensor_scalar_mul(
                            dst[:, nt, c0 + 512:c0 + D], ps2[:, 0:256],
                            C_U)

            def combine(pair, u8, u0b):
                """out = U0b + sum_m S^m @ U_m; one PSUM group per tile."""
                for nt in range(NT):
                    ot = ostp.tile([128, PW], F32, name=f"o{pair}_{nt}",
                                   tag="ost")
                    last = (pair == 1 and nt == NT - 1)
                    b0 = 2 * pair
                    r0 = [(b0 + bi) * N + nt * 128 for bi in range(2)]
                    for ck in range(PW // 512):
                        ps = psH.tile([128, 512], F32,
                                      name=f"pH{pair}{nt}{ck}", tag="ps")
                        first = True
                        for pm in range(3):
                            for t in range(NT // 2):
                                nc.tensor.matmul(
                                    ps[:],
                                    S8s[:, pm, 2 * t:2 * t + 2,
                                        nt * 128:(nt + 1) * 128],
                                    u8[pm][:, 2 * t:2 * t + 2,
                                           ck * 512:(ck + 1) * 512],
                                    start=first,
                                    stop=(pm == 2 and t == 1),
                                    perf_mode=DR)
                                first = False
                        nc.vector.scalar_tensor_tensor(
                            ot[:, ck * 512:(ck + 1) * 512],
                            ps[:], C_O,
                            u0b[:, nt, ck * 512:(ck + 1) * 512],
                            ALU.mult, ALU.add)
                        if last:
                            # final tile: store each chunk as soon as its
                            # STT lands, finishing with two 131KB pieces
                            # in parallel so the end receipt chain is short
                            if ck == 0:
                                nc.sync.dma_start(
                                    out_d[r0[0]:r0[0] + 128, 0:512],
                                    ot[:, 0:512])
                            elif ck == 1:
                                nc.scalar.dma_start(
                                    out_d[r0[0]:r0[0] + 128, 512:D],
                                    ot[:, 512:D])
                                nc.sync.dma_start(
                                    out_d[r0[1]:r0[1] + 128, 0:256],
                                    ot[:, D:D + 256])
                            else:
                                nc.scalar.dma_start(
                                    out_d[r0[1]:r0[1] + 128, 256:512],
                                    ot[:, D + 256:D + 512])
                                nc.sync.dma_start(
                                    out_d[r0[1]:r0[1] + 128, 512:D],
                                    ot[:, D + 512:PW])
                    if not last:
                        for bi in range(2):
                            # pair 1 keeps gpsimd idle so its SWDGE ring
                            # drain (~3us) overlaps compute, not the tail
                            engs = ((nc.gpsimd, nc.sync),
                                    (nc.scalar, nc.sync))[pair]
                            engs[(2 * nt + bi) % 2].dma_start(
                                out_d[r0[bi]:r0[bi] + 128, :],
                                ot[:, bi * D:(bi + 1) * D])

            # ---- schedule ----
            def mk_u8(pair):
                # index pm: 0 -> U1, 1 -> U2, 2 -> U3
                return [utp.tile([128, NT, PW], F8,
                                 name=f"u8_{pair}_{pm}", tag="u")
                        for pm in range(3)]

            u8_0, u8_1 = mk_u8(0), mk_u8(1)

            proj_um(0, 3, u8_0[2])
            proj_um(0, 2, u8_0[1])
            proj_um(0, 1, u8_0[0])
            u0b_0 = proj_u0(0)
            combine(0, u8_0, u0b_0)

            proj_um(1, 3, u8_1[2])
            proj_um(1, 2, u8_1[1])
            proj_um(1, 1, u8_1[0])
            u0b_1 = proj_u0(1)
            combine(1, u8_1, u0b_1)
    nc.compile()
    return nc


_CACHE = {}


def _get_program():
    if "nc" not in _CACHE:
        _CACHE["nc"] = _build_program()
    return _CACHE["nc"]


def make_in_maps(inputs, adj, weights, biases):
    inputs = np.ascontiguousarray(inputs, dtype=np.float32)
    adj = np.ascontiguousarray(adj, dtype=np.float32)
    weights = np.ascontiguousarray(weights, dtype=np.float32)
    biases = np.ascontiguousarray(biases, dtype=np.float32)
    assert inputs.shape == (B, N, D)
    assert adj.shape == (N, N)
    assert weights.shape == (D * 4, D)
    assert biases.shape == (D,)

    def perm(a, kt):
        # [kt*128, F] -> [128, kt, F] partition-major tile image
        F = a.shape[1]
        return np.ascontiguousarray(
            a.reshape(kt, 128, F).transpose(1, 0, 2))

    # support S = D^-1/2 (adj+I)^T D^-1/2; powers in fp32, then fp8.
    # lhsT layout needs (S^m)^T = (S^T)^m.
    m = adj + np.eye(N, dtype=np.float32)
    dd = m.sum(axis=1) ** -0.5
    ST = np.ascontiguousarray(
        ((m * dd[None, :]).T * dd[None, :]).astype(np.float32).T)
    ST2 = (ST @ ST).astype(np.float32)
    ST3 = (ST2 @ ST).astype(np.float32)
    # [128, 3, NT, N] partition-major
    s8P = np.ascontiguousarray(np.stack(
        [perm((p * SS).astype(E4M3), NT) for p in (ST, ST2, ST3)]
    ).transpose(1, 0, 2, 3))

    W4 = weights.reshape(D, 4, D)
    v0P = perm(np.ascontiguousarray(W4[:, 0] - W4[:, 2])
               .astype(ml_dtypes.bfloat16), DT)       # [128, DT, 768]
    v0A = np.ascontiguousarray(v0P[:, :, 0:512])
    v0B = np.ascontiguousarray(v0P[:, :, 512:768])
    # [128, 3, DT, 768] partition-major, then column-split
    v8P = np.ascontiguousarray(np.stack([
        perm(np.ascontiguousarray(v * SV).astype(E4M3), DT)
        for v in (W4[:, 1] - 3.0 * W4[:, 3], 2.0 * W4[:, 2],
                  4.0 * W4[:, 3])]).transpose(1, 0, 2, 3))
    v8A = np.ascontiguousarray(v8P[:, :, :, 0:512])
    v8B = np.ascontiguousarray(v8P[:, :, :, 512:768])
    biasb = np.ascontiguousarray(
        np.broadcast_to(biases[None, :], (128, D)).astype(np.float32))

    in_maps = []
    for c in range(N_CORES):
        xc = inputs[c * BL:(c + 1) * BL].reshape(BN, D)
        # x0R[p, rt, t, r] = xc[rt*128+r, t*128+p]
        x0R = np.ascontiguousarray(
            xc.reshape(RT, 128, DT, 128).transpose(3, 0, 2, 1)
            .astype(ml_dtypes.bfloat16))
        # x8P[p, rt, t, r] = 8*xc[rt*128+r, t*128+p] quantized
        x8P = np.ascontiguousarray(
            (xc * SX).reshape(RT, 128, DT, 128)
            .transpose(3, 0, 2, 1).astype(E4M3))
        in_maps.append({
            "x0R": x0R,
            "x8P": x8P,
            "v0A": v0A,
            "v0B": v0B,
            "v8A": v8A,
            "v8B": v8B,
            "s8P": s8P,
            "bias": biasb,
        })
    return in_maps


def kernel(inputs, adj, weights, biases):
    nc = _get_program()
    in_maps = make_in_maps(inputs, adj, weights, biases)
    res = run_bass_kernel_spmd(nc, in_maps, list(range(N_CORES)))
    out = np.concatenate(
        [res.results[c]["out"].reshape(BL, N, D) for c in range(N_CORES)],
        axis=0)
    return out


# revision 43
# speedup vs baseline: 1.0069x; 1.0069x over previous
"""DGCN diffusion-graph-conv kernel for 8 Trainium2 NeuronCores.

Math (per the reference):
    support S = D^-1/2 (adj+I)^T D^-1/2  with D = diag(rowsum(adj+I))
    x_m = T_m(S) x0  (Chebyshev recurrence, K=3 -> m=0..3)
    out = sum_m x_m @ W_m + bias

Strategy (data-parallel over batch, 4 batches/core, mixed precision):
    Fold Chebyshev coefficients into the weights and expand the
    recurrence into explicit support powers (host-precomputed in fp32):
        V0 = W0 - W2, V1 = W1 - 3*W3, V2 = 2*W2, V3 = 4*W3
        U_m = x0 @ V_m                        (contracts feature dim)
        out = U0 + S*U1 + S^2*U2 + S^3*U3 + bias   (contracts nodes)
    U0 feeds the output undamped -> bf16 matmuls (fp8 measured 4e-2,
    over the gate).  U1..U3 and the S^m multiplies are damped by the
    contractive support spectrum -> fp8 e4m3 DoubleRow matmuls (2x PE
    rate).  The three S^m terms accumulate into one PSUM group, so
    there is no serial diffusion chain on the device at all.
    Power-of-2 scales keep fp8 operands in the normal range:
        x*8, V_m*32 (m>=1), S^m*2^14, U_m carried *8.

Schedule notes (v3): every DRAM image is partition-major AND column-
split so each startup DMA moves contiguous-per-partition lines (the
strided 512B-segment loads measured ~22 GB/s; contiguous ~300+);
startup DMAs ordered critical-first across the three DMA queues
(sync/scalar HWDGE, gpsimd SWDGE); x0R persistent; psum->fp8 converts
split between scalar ACT (512 cols) and vector DVE (256 cols); junk
matmuls pre-warm the HAM clock gate during the initial DMA wait.
"""

import numpy as np
import ml_dtypes

import concourse.bacc as bacc
import concourse.tile as tile
import concourse.mybir as mybir
from concourse.bass_utils import run_bass_kernel_spmd

F32 = mybir.dt.float32
BF16 = mybir.dt.bfloat16
F8 = mybir.dt.float8e4
DR = mybir.MatmulPerfMode.DoubleRow
ALU = mybir.AluOpType
AFT = mybir.ActivationFunctionType
E4M3 = ml_dtypes.float8_e4m3

N_CORES = 8
B, N, D = 32, 512, 768
BL = B // N_CORES          # local batches per core = 4
BN = BL * N                # local rows = 2048
RT = BN // 128             # 16 row tiles
NT = N // 128              # 4 node tiles per batch
DT = D // 128              # 6 feature k-subtiles
PW = 1536                  # batch-pair column width (2*768)
N_WARM = 13                # junk matmuls to pre-warm the PE clock

SX = 8.0                   # x fp8 pre-scale
SV = 32.0                  # V1..V3 fp8 pre-scale
SS = float(2 ** 14)        # S^m fp8 pre-scale
SU = 8.0                   # U_m carried *8 in fp8
C_U = SU / (SX * SV)       # psum(U123) -> 8*U_m     (= 1/32)
C_O = 1.0 / (SS * SU)      # psum(combine) -> out    (= 2^-17)


def _build_program():
    nc = bacc.Bacc("TRN2", target_bir_lowering=False, debug=False,
                   num_devices=N_CORES)
    # Partition-major images; V matrices pre-split into the 512- and
    # 256-wide column groups so every DMA is contiguous per partition.
    x0R_d = nc.dram_tensor("x0R", [128, RT, DT, 128], BF16,
                           kind="ExternalInput").ap()
    x8P_d = nc.dram_tensor("x8P", [128, RT, DT, 128], F8,
                           kind="ExternalInput").ap()
    v0A_d = nc.dram_tensor("v0A", [128, DT, 512], BF16,
                           kind="ExternalInput").ap()
    v0B_d = nc.dram_tensor("v0B", [128, DT, 256], BF16,
                           kind="ExternalInput").ap()
    v8A_d = nc.dram_tensor("v8A", [128, 3, DT, 512], F8,
                           kind="ExternalInput").ap()
    v8B_d = nc.dram_tensor("v8B", [128, 3, DT, 256], F8,
                           kind="ExternalInput").ap()
    s8P_d = nc.dram_tensor("s8P", [128, 3, NT, N], F8,
                           kind="ExternalInput").ap()
    # bias pre-broadcast on host so no SWDGE replicate DMA is needed
    bias_d = nc.dram_tensor("bias", [128, D], F32,
                            kind="ExternalInput").ap()
    out_d = nc.dram_tensor("out", [BN, D], F32, kind="ExternalOutput").ap()

    with tile.TileContext(nc) as tc:
        with (
            tc.tile_pool(name="const", bufs=1) as constp,
            tc.tile_pool(name="ut", bufs=6) as utp,
            tc.tile_pool(name="u0t", bufs=2) as u0p,
            tc.tile_pool(name="ost", bufs=3) as ostp,
            tc.tile_pool(name="psP", bufs=6, space="PSUM") as psP,
            tc.tile_pool(name="psH", bufs=2, space="PSUM") as psH,
        ):
            X8 = constp.tile([128, RT, DT, 128], F8, name="X8")
            X0R = constp.tile([128, RT, DT, 128], BF16, name="X0R")
            V8A = constp.tile([128, 3, DT, 512], F8, name="V8A")
            V8B = constp.tile([128, 3, DT, 256], F8, name="V8B")
            V0A = constp.tile([128, DT, 512], BF16, name="V0A")
            V0B = constp.tile([128, DT, 256], BF16, name="V0B")
            S8s = constp.tile([128, 3, NT, N], F8, name="S8s")
            biasb = constp.tile([128, D], F32, name="biasb")
            warm = constp.tile([128, 512], BF16, name="warm")

            # ---- startup DMAs ----
            # The um(0,3) phase re-streams ALL of V8[2] within its first
            # row-tile (~1us in), so V8[2] + X8[0] gate the start.  Put
            # the critical stream on BOTH HWDGE rings interleaved in
            # deadline order; everything not needed before ~40us goes on
            # the delayed gpsimd ring or the HWDGE tails.
            # scalar issues only 4 chunks — its queue must drain before
            # the first psum->fp8 ACT (~11us) or the ACTs (and the psum
            # recycling behind them) block on DMA lane-reuse waits.
            nc.scalar.dma_start(V8A[:, 2, 0:2, :], v8A_d[:, 2, 0:2, :])
            nc.sync.dma_start(X8[:, 0:1, :, :], x8P_d[:, 0:1, :, :])
            nc.scalar.dma_start(V8A[:, 2, 2:6, :], v8A_d[:, 2, 2:6, :])
            nc.sync.dma_start(V8B[:, 2, :, :], v8B_d[:, 2, :, :])
            nc.scalar.dma_start(V8B[:, 1, :, :], v8B_d[:, 1, :, :])
            nc.sync.dma_start(X8[:, 1:2, :, :], x8P_d[:, 1:2, :, :])
            nc.sync.dma_start(X8[:, 2:4, :, :], x8P_d[:, 2:4, :, :])
            nc.sync.dma_start(X8[:, 4:8, :, :], x8P_d[:, 4:8, :, :])
            nc.sync.dma_start(V8A[:, 1, :, :], v8A_d[:, 1, :, :])
            nc.sync.dma_start(V8A[:, 0, :, :], v8A_d[:, 0, :, :])
            nc.sync.dma_start(V8B[:, 0, :, :], v8B_d[:, 0, :, :])
            nc.sync.dma_start(S8s[:, :, :, :], s8P_d[:, :, :, :])
            nc.sync.dma_start(X8[:, 8:12, :, :], x8P_d[:, 8:12, :, :])
            nc.sync.dma_start(X8[:, 12:16, :, :], x8P_d[:, 12:16, :, :])
            nc.sync.dma_start(X0R[:, 8:12, :, :], x0R_d[:, 8:12, :, :])
            nc.sync.dma_start(X0R[:, 12:16, :, :], x0R_d[:, 12:16, :, :])
            # gpsimd (SWDGE): its bulk is not needed before ~40us, so
            # gate each DMA behind a junk-memset wall via tiny copies
            # (copy reads the stall tile -> RAW dep; DMA dst overlaps
            # the copy's dst -> WAW dep).  The scheduler cannot hoist
            # these DMAs into the critical 8-13us window.
            stall = constp.tile([128, 2048], F32, name="stall")
            nc.gpsimd.memset(stall[:], 0.0)
            nc.gpsimd.tensor_scalar_add(biasb[0:128, 0:4], stall[:, 0:4], 0.0)
            nc.gpsimd.memset(stall[:], 1.0)
            nc.gpsimd.memset(stall[:], 2.0)
            nc.gpsimd.tensor_scalar_add(V0A[0:128, 0, 0:4], stall[:, 0:4], 0.0)
            nc.gpsimd.tensor_scalar_add(V0B[0:128, 0, 0:4], stall[:, 4:8], 0.0)
            nc.gpsimd.memset(stall[:], 3.0)
            nc.gpsimd.tensor_scalar_add(
                X0R[0:128, 0, 0, 0:4], stall[:, 0:4], 0.0)
            nc.gpsimd.tensor_scalar_add(
                X0R[0:128, 4, 0, 0:4], stall[:, 4:8], 0.0)
            nc.gpsimd.dma_start(biasb[:], bias_d[:, :])
            nc.gpsimd.dma_start(V0A[:, :, :], v0A_d[:, :, :])
            nc.gpsimd.dma_start(V0B[:, :, :], v0B_d[:, :, :])
            nc.gpsimd.dma_start(X0R[:, 0:4, :, :], x0R_d[:, 0:4, :, :])
            nc.gpsimd.dma_start(X0R[:, 4:8, :, :], x0R_d[:, 4:8, :, :])

            # PE warm-up: junk matmuls during the initial DMA wait ramp
            # the tensor-engine p-state so real work starts at full
            # clock.  They use the psH pool (idle until combine, ~49us)
            # so the warm chain never blocks the real stream's psP ring.
            nc.vector.memset(warm[:], 0.0)
            for i in range(N_WARM):
                pw = psH.tile([128, 512], F32, name=f"warm{i}", tag="ps")
                nc.tensor.matmul(pw[:], warm[:, 0:128], warm[:],
                                 start=True, stop=True)

            def proj_u0(pair):
                """U0 (+bias) for batch pair (bf16 matmuls) -> u0b fp32."""
                u0b = u0p.tile([128, NT, PW], F32, name=f"u0b{pair}",
                               tag="u0")
                for bi in range(2):
                    b = 2 * pair + bi
                    for nt in range(NT):
                        rt = b * NT + nt
                        ps = psP.tile([128, 512], F32, name=f"pA{rt}",
                                      tag="ps")
                        ps2 = psP.tile([128, 512], F32, name=f"pB{rt}",
                                       tag="ps")
                        for t in range(DT):
                            nc.tensor.matmul(
                                ps[:], X0R[:, rt, t, :], V0A[:, t, :],
                                start=(t == 0), stop=(t == DT - 1))
                            nc.tensor.matmul(
                                ps2[:, 0:256], X0R[:, rt, t, :],
                                V0B[:, t, :],
                                start=(t == 0), stop=(t == DT - 1))
                        c0 = bi * D
                        nc.vector.tensor_add(
                            u0b[:, nt, c0:c0 + 512], ps[:], biasb[:, 0:512])
                        nc.vector.tensor_add(
                            u0b[:, nt, c0 + 512:c0 + D], ps2[:, 0:256],
                            biasb[:, 512:D])
                return u0b

            def proj_um(pair, m, dst):
                """One U_m phase (fp8 DoubleRow) for a batch pair -> fp8.

                psum->fp8 converts split: 512-chunk on scalar ACT,
                256-chunk on vector DVE, so neither engine paces the PE.
                """
                for bi in range(2):
                    b = 2 * pair + bi
                    for nt in range(NT):
                        rt = b * NT + nt
                        c0 = bi * D
                        ps = psP.tile([128, 512], F32,
                                      name=f"pU{rt}_{m}", tag="ps")
                        ps2 = psP.tile([128, 512], F32,
                                       name=f"pV{rt}_{m}", tag="ps")
                        for t in range(DT // 2):
                            xs = X8[:, rt, 2 * t:2 * t + 2, :]
                            nc.tensor.matmul(
                                ps[:], xs,
                                V8A[:, m - 1, 2 * t:2 * t + 2, :],
                                start=(t == 0), stop=(t == 2),
                                perf_mode=DR)
                            nc.tensor.matmul(
                                ps2[:, 0:256], xs,
                                V8B[:, m - 1, 2 * t:2 * t + 2, :],
                                start=(t == 0), stop=(t == 2),
                                perf_mode=DR)
                        nc.scalar.activation(
                            dst[:, nt, c0:c0 + 512], ps[:],
                            AFT.Copy, scale=C_U)
                        nc.vector.tensor_scalar_mul(
                            dst[:, nt, c0 + 512:c0 + D], ps2[:, 0:256],
                            C_U)

            def combine(pair, u8, u0b):
                """out = U0b + sum_m S^m @ U_m; one PSUM group per tile."""
                for nt in range(NT):
                    ot = ostp.tile([128, PW], F32, name=f"o{pair}_{nt}",
                                   tag="ost")
                    last = (pair == 1 and nt == NT - 1)
                    b0 = 2 * pair
                    r0 = [(b0 + bi) * N + nt * 128 for bi in range(2)]
                    for ck in range(PW // 512):
                        ps = psH.tile([128, 512], F32,
                                      name=f"pH{pair}{nt}{ck}", tag="ps")
                        first = True
                        for pm in range(3):
                            for t in range(NT // 2):
                                nc.tensor.matmul(
                                    ps[:],
                                    S8s[:, pm, 2 * t:2 * t + 2,
                                        nt * 128:(nt + 1) * 128],
                                    u8[pm][:, 2 * t:2 * t + 2,
                                           ck * 512:(ck + 1) * 512],
                                    start=first,
                                    stop=(pm == 2 and t == 1),
                                    perf_mode=DR)
                                first = False
                        nc.vector.scalar_tensor_tensor(
                            ot[:, ck * 512:(ck + 1) * 512],
                            ps[:], C_O,
                            u0b[:, nt, ck * 512:(ck + 1) * 512],
                            ALU.mult, ALU.add)
                        if last:
                            # final tile: store each chunk as soon as its
                            # STT lands, finishing with two 131KB pieces
                            # in parallel so the end receipt chain is short
                            if ck == 0:
                                nc.sync.dma_start(
                                    out_d[r0[0]:r0[0] + 128, 0:512],
                                    ot[:, 0:512])
                            elif ck == 1:
                                nc.scalar.dma_start(
                                    out_d[r0[0]:r0[0] + 128, 512:D],
                                    ot[:, 512:D])
                                nc.sync.dma_start(
                                    out_d[r0[1]:r0[1] + 128, 0:256],
                                    ot[:, D:D + 256])
                            else:
                                nc.scalar.dma_start(
                                    out_d[r0[1]:r0[1] + 128, 256:512],
                                    ot[:, D + 256:D + 512])
                                nc.sync.dma_start(
                                    out_d[r0[1]:r0[1] + 128, 512:D],
                                    ot[:, D + 512:PW])
                    if not last:
                        for bi in range(2):
                            # pair 1 keeps gpsimd idle so its SWDGE ring
                            # drain (~3us) overlaps compute, not the tail
                            engs = ((nc.gpsimd, nc.sync),
                                    (nc.scalar, nc.sync))[pair]
                            engs[(2 * nt + bi) % 2].dma_start(
                                out_d[r0[bi]:r0[bi] + 128, :],
                                ot[:, bi * D:(bi + 1) * D])

            # ---- schedule ----
            def mk_u8(pair):
                # index pm: 0 -> U1, 1 -> U2, 2 -> U3
                return [utp.tile([128, NT, PW], F8,
                                 name=f"u8_{pair}_{pm}", tag="u")
                        for pm in range(3)]

            u8_0, u8_1 = mk_u8(0), mk_u8(1)

            proj_um(0, 3, u8_0[2])
            proj_um(0, 2, u8_0[1])
            proj_um(0, 1, u8_0[0])
            u0b_0 = proj_u0(0)
            combine(0, u8_0, u0b_0)

            proj_um(1, 3, u8_1[2])
            proj_um(1, 2, u8_1[1])
            proj_um(1, 1, u8_1[0])
            u0b_1 = proj_u0(1)
            combine(1, u8_1, u0b_1)
    nc.compile()
    return nc


_CACHE = {}


def _get_program():
    if "nc" not in _CACHE:
        _CACHE["nc"] = _build_program()
    return _CACHE["nc"]


def make_in_maps(inputs, adj, weights, biases):
    inputs = np.ascontiguousarray(inputs, dtype=np.float32)
    adj = np.ascontiguousarray(adj, dtype=np.float32)
    weights = np.ascontiguousarray(weights, dtype=np.float32)
    biases = np.ascontiguousarray(biases, dtype=np.float32)
    assert inputs.shape == (B, N, D)
    assert adj.shape == (N, N)
    assert weights.shape == (D * 4, D)
    assert biases.shape == (D,)

    def perm(a, kt):
        # [kt*128, F] -> [128, kt, F] partition-major tile image
        F = a.shape[1]
        return np.ascontiguousarray(
            a.reshape(kt, 128, F).transpose(1, 0, 2))

    # support S = D^-1/2 (adj+I)^T D^-1/2; powers in fp32, then fp8.
    # lhsT layout needs (S^m)^T = (S^T)^m.
    m = adj + np.eye(N, dtype=np.float32)
    dd = m.sum(axis=1) ** -0.5
    ST = np.ascontiguousarray(
        ((m * dd[None, :]).T * dd[None, :]).astype(np.float32).T)
    ST2 = (ST @ ST).astype(np.float32)
    ST3 = (ST2 @ ST).astype(np.float32)
    # [128, 3, NT, N] partition-major
    s8P = np.ascontiguousarray(np.stack(
        [perm((p * SS).astype(E4M3), NT) for p in (ST, ST2, ST3)]
    ).transpose(1, 0, 2, 3))

    W4 = weights.reshape(D, 4, D)
    v0P = perm(np.ascontiguousarray(W4[:, 0] - W4[:, 2])
               .astype(ml_dtypes.bfloat16), DT)       # [128, DT, 768]
    v0A = np.ascontiguousarray(v0P[:, :, 0:512])
    v0B = np.ascontiguousarray(v0P[:, :, 512:768])
    # [128, 3, DT, 768] partition-major, then column-split
    v8P = np.ascontiguousarray(np.stack([
        perm(np.ascontiguousarray(v * SV).astype(E4M3), DT)
        for v in (W4[:, 1] - 3.0 * W4[:, 3], 2.0 * W4[:, 2],
                  4.0 * W4[:, 3])]).transpose(1, 0, 2, 3))
    v8A = np.ascontiguousarray(v8P[:, :, :, 0:512])
    v8B = np.ascontiguousarray(v8P[:, :, :, 512:768])
    biasb = np.ascontiguousarray(
        np.broadcast_to(biases[None, :], (128, D)).astype(np.float32))

    in_maps = []
    for c in range(N_CORES):
        xc = inputs[c * BL:(c + 1) * BL].reshape(BN, D)
        # x0R[p, rt, t, r] = xc[rt*128+r, t*128+p]
        x0R = np.ascontiguousarray(
            xc.reshape(RT, 128, DT, 128).transpose(3, 0, 2, 1)
            .astype(ml_dtypes.bfloat16))
        # x8P[p, rt, t, r] = 8*xc[rt*128+r, t*128+p] quantized
        x8P = np.ascontiguousarray(
            (xc * SX).reshape(RT, 128, DT, 128)
            .transpose(3, 0, 2, 1).astype(E4M3))
        in_maps.append({
            "x0R": x0R,
            "x8P": x8P,
            "v0A": v0A,
            "v0B": v0B,
            "v8A": v8A,
            "v8B": v8B,
            "s8P": s8P,
            "bias": biasb,
        })
    return in_maps


def kernel(inputs, adj, weights, biases):
    nc = _get_program()
    in_maps = make_in_maps(inputs, adj, weights, biases)
    res = run_bass_kernel_spmd(nc, in_maps, list(range(N_CORES)))
    out = np.concatenate(
        [res.results[c]["out"].reshape(BL, N, D) for c in range(N_CORES)],
        axis=0)
    return out
